# revision 5
# baseline (speedup 1.0000x reference)
"""Trainium2 Bass kernel for nn_MultiHeadAttention_54211077210696.

8-core SPMD sharding: batch (2-way) x heads (4-way).
Core c (b = c//4, j = c%4) computes heads 4j..4j+3 of batch b.

v2 design:
  - Q^T/K^T projections in head-pair layout [128, S] (head h of pair at
    partitions h*64..h*64+64) -- no zero padding.
  - Scores via 64-contraction matmuls, row-tiled: the two heads of a pair
    run CONCURRENTLY on the PE array (tile_position (0,0)/(64,0) derived
    from base partitions), halving score time.
  - exp on ScalarE in wide instructions: PSUM slot A [128,2048] holds 2
    kt-tiles' scores (2 heads x 512q each), slot B [128,1024] holds 1.
    Pattern A,B,A,B,... decouples scores(kt+1) from exp(kt) (no WAR stall)
    while 2/3 of exp columns go through 2048-wide activations.
  - Softmax denominator from an all-ones block appended to V (rows 64..127
    of the PV output are the denominator).
  - q processed in 4 chunks of 512; out-projection + fp16 ReduceScatter
    fired per chunk, interleaved into the next chunk's score sweep so the
    collective tail is a single 1MB RS + one LayerNorm.
  - All LayerNorms deferred past the last exp (single ACT table switch).
  - Inputs prefetched on 5 DMA queues (sync/scalar/vector/tensor/gpsimd).
Matmuls fp16 with fp32 PSUM accumulation.
"""

import numpy as np
from contextlib import ExitStack

import concourse.bass as bass  # noqa: F401  (registers bass types)
import concourse.tile as tile
from concourse import bacc, mybir
from concourse.bass_utils import run_bass_kernel_spmd

F32 = mybir.dt.float32
F16 = mybir.dt.float16
AF = mybir.ActivationFunctionType
ALU = mybir.AluOpType

_NC = None

N_CORES = 8
GROUPS = [[0, 1, 2, 3], [4, 5, 6, 7]]
B, S, DM = 2, 2048, 1024
HL = 4          # heads per core
DT = 2          # head pairs per core
KT16 = 16       # 128-row k tiles
NCH = 4         # q chunks
QC = 512        # q chunk width
EPS = 1e-5


def _slot_plan():
    """Slot pattern for one (chunk, pair) sweep: [('A',[0,1]),('B',[2]),...]"""
    plan, k = [], 0
    while k < KT16:
        if len(plan) % 2 == 0:
            n = min(2, KT16 - k)
        else:
            n = 1
        plan.append(("AB"[len(plan) % 2], list(range(k, k + n))))
        k += n
    return plan


def _layernorm_c(nc, t, c, lnp, stp, resid_sb, eps_t):
    ro16 = lnp.tile([128, DM], F16, tag="ro16", name="ro16")
    nc.gpsimd.dma_start(ro16[:], t["rs_out"][c].ap())
    of = lnp.tile([128, DM], F32, tag="of", name="of")
    nc.vector.tensor_copy(of[:], ro16[:])
    orow = lnp.tile([128, DM], F32, tag="orow", name="orow")
    nc.vector.tensor_add(orow[:], of[:], resid_sb[c][:])
    stats = stp.tile([128, 2, 6], F32, tag="st", name="st")
    for i in range(2):
        nc.vector.bn_stats(stats[:, i, :], orow[:, i * 512:(i + 1) * 512])
    mv = stp.tile([128, 2], F32, tag="mv", name="mv")
    nc.vector.bn_aggr(mv[:], stats[:])
    rstd = stp.tile([128, 1], F32, tag="rstd", name="rstd")
    nc.scalar.activation(rstd[:], mv[:, 1:2], AF.Sqrt, bias=eps_t[:], scale=1.0)
    nc.vector.reciprocal(rstd[:], rstd[:])
    normed = lnp.tile([128, DM], F32, tag="norm", name="norm")
    nc.vector.tensor_scalar(normed[:], orow[:], mv[:, 0:1], rstd[:],
                            ALU.subtract, ALU.mult)
    nc.sync.dma_start(t["out"][c][:, 0:512], normed[:, 0:512])
    nc.scalar.dma_start(t["out"][c][:, 512:1024], normed[:, 512:1024])


def _emit(nc, tc, ctx, t):
    sing = ctx.enter_context(tc.tile_pool(name="sing", bufs=1))
    xp = ctx.enter_context(tc.tile_pool(name="xp", bufs=4))
    eop = ctx.enter_context(tc.tile_pool(name="eop", bufs=2))
    ctp = ctx.enter_context(tc.tile_pool(name="ctp", bufs=4))
    rp = ctx.enter_context(tc.tile_pool(name="rp", bufs=2))
    ostp = ctx.enter_context(tc.tile_pool(name="ostp", bufs=2))
    lnp = ctx.enter_context(tc.tile_pool(name="lnp", bufs=2))
    stp = ctx.enter_context(tc.tile_pool(name="stp", bufs=4))
    psp = ctx.enter_context(tc.tile_pool(name="psp", bufs=1, space="PSUM"))

    # persistent SBUF tiles
    QTt = [sing.tile([128, S], F16, tag=f"qt{d}", name=f"qt{d}") for d in range(DT)]
    KTt = [sing.tile([128, S], F16, tag=f"kt{d}", name=f"kt{d}") for d in range(DT)]
    vaug = sing.tile([128, HL * KT16 * 128], F16, tag="vaug")
    wo_sb = [sing.tile([128, DM], F16, tag=f"wo{p}", name=f"wo{p}") for p in range(DT)]
    resid_sb = [sing.tile([128, DM], F32, tag=f"res{c}", name=f"res{c}")
                for c in range(NCH)]
    eps_t = sing.tile([128, 1], F32, tag="eps")
    wkt = [sing.tile([128, 256], F16, tag=f"wk{i}", name=f"wk{i}") for i in range(8)]
    wqt = [sing.tile([128, 256], F16, tag=f"wq{i}", name=f"wq{i}") for i in range(8)]
    wvt = [sing.tile([128, 256], F16, tag=f"wv{i}", name=f"wv{i}") for i in range(8)]
    xv_res = [sing.tile([128, S], F16, tag=f"xv{i}", name=f"xv{i}") for i in range(8)]

    nc.vector.memset(eps_t[:], EPS)
    warm = sing.tile([128, 1], F32, tag="warm")
    nc.scalar.activation(warm[:], eps_t[:], AF.Exp, scale=1.0)
    nc.gpsimd.memset(vaug[:], 1.0)

    # ---- DMA prefetch on the 3 HWDGE queues (sync/scalar/gpsimd) ----
    for i in range(8):
        nc.sync.dma_start(wkt[i][:], t["wk"][i * 128:(i + 1) * 128, :])
    for i in range(8):
        nc.gpsimd.dma_start(wqt[i][:], t["wq"][i * 128:(i + 1) * 128, :])
    for i in range(8):
        nc.gpsimd.dma_start(wvt[i][:], t["wv"][i * 128:(i + 1) * 128, :])
    for p in range(DT):
        nc.gpsimd.dma_start(wo_sb[p][:], t["wo"][p * 128:(p + 1) * 128, :])
    for i in range(8):
        nc.gpsimd.dma_start(xv_res[i][:, 0:1024],
                            t["xvT"][i * 128:(i + 1) * 128, 0:1024])
        nc.gpsimd.dma_start(xv_res[i][:, 1024:2048],
                            t["xvT"][i * 128:(i + 1) * 128, 1024:2048])
    for c in range(NCH):
        nc.gpsimd.dma_start(resid_sb[c][:], t["resid"][c])

    # ---- K / Q projections: both d-tiles accumulate across all 8 banks ----
    # d0 -> A (4 chains), d1 -> B (2 chains) + 2 cg chains; all bank-aligned.
    def qk_sweep(x_dram, wt, dst, xtag, q1, q2):
        psA = psp.tile([128, 2048], F32, tag="A", name="psA")
        psB = psp.tile([128, 1024], F32, tag="Bp", name="psB")
        psc = [psp.tile([128, 512], F32, tag="cg", name="psc", bufs=2) for _ in range(2)]
        for dmc in range(8):
            xc = xp.tile([128, S], F16, tag=xtag, name="xc")
            q1.dma_start(xc[:, 0:1024], x_dram[dmc * 128:(dmc + 1) * 128, 0:1024])
            q2.dma_start(xc[:, 1024:2048], x_dram[dmc * 128:(dmc + 1) * 128, 1024:2048])
            st = dict(start=(dmc == 0), stop=(dmc == 7))
            for sc in range(4):
                nc.tensor.matmul(psA[:, sc * 512:(sc + 1) * 512],
                                 wt[dmc][:, 0:128], xc[:, sc * 512:(sc + 1) * 512], **st)
            for sc in range(4):
                out = (psB[:, sc * 512:(sc + 1) * 512] if sc < 2
                       else psc[sc - 2][:, :])
                nc.tensor.matmul(out, wt[dmc][:, 128:256],
                                 xc[:, sc * 512:(sc + 1) * 512], **st)
        for sc in range(4):
            nc.vector.tensor_copy(dst[0][:, sc * 512:(sc + 1) * 512],
                                  psA[:, sc * 512:(sc + 1) * 512])
        for sc in range(2):
            nc.vector.tensor_copy(dst[1][:, sc * 512:(sc + 1) * 512],
                                  psB[:, sc * 512:(sc + 1) * 512])
        for i in range(2):
            nc.vector.tensor_copy(dst[1][:, (2 + i) * 512:(3 + i) * 512], psc[i][:])

    qk_sweep(t["xkT"], wkt, KTt, "xk", nc.sync, nc.scalar)
    qk_sweep(t["xqT"], wqt, QTt, "xq", nc.sync, nc.scalar)

    # ---- V projection: 2 waves x 8 s-tile chains across all 8 banks ----
    for wave in range(2):
        psA = psp.tile([128, 2048], F32, tag="A", name="psAv")
        psB = psp.tile([128, 1024], F32, tag="Bp", name="psBv")
        psc = [psp.tile([128, 512], F32, tag="cg", name="pscv", bufs=2) for _ in range(2)]

        def vchain(i):
            if i < 4:
                return psA[:, i * 512:i * 512 + 256]
            if i < 6:
                return psB[:, (i - 4) * 512:(i - 4) * 512 + 256]
            return psc[i - 6][:, 0:256]

        for dmc in range(8):
            for i in range(8):
                st = wave * 8 + i
                nc.tensor.matmul(vchain(i), xv_res[dmc][:, st * 128:(st + 1) * 128],
                                 wvt[dmc][:], start=(dmc == 0), stop=(dmc == 7))
        for i in range(8):
            st = wave * 8 + i
            src = vchain(i)
            for h in range(HL):
                nc.vector.tensor_copy(
                    vaug[:, (h * KT16 + st) * 128:(h * KT16 + st) * 128 + 64],
                    src[:, h * 64:h * 64 + 64])

    # ---- attention: 4 chunks x 2 pairs, slot pattern A(2kt) B(1kt) ----
    plan = _slot_plan()

    def emit_pv(p, caug, eo, kts):
        for i, kt in enumerate(kts):
            for h in range(2):
                blk = ((p * 2 + h) * KT16 + kt) * 128
                nc.tensor.matmul(caug[h][:, :], vaug[:, blk:blk + 128],
                                 eo[:, i * 1024 + h * 512:i * 1024 + (h + 1) * 512],
                                 start=(kt == 0), stop=(kt == KT16 - 1))

    def sweep(c, p, hooks):
        qb = c * QC
        caug = [psp.tile([128, QC], F32, tag="cg", name=f"caug{h}", bufs=2) for h in range(2)]
        pend = None
        hooks = dict(hooks)
        for si, (kind, kts) in enumerate(plan):
            w = 1024 * len(kts)
            ps = psp.tile([128, 2048 if kind == "A" else 1024], F32,
                          tag=("A" if kind == "A" else "Bp"), name="pss")
            for i, kt in enumerate(kts):
                co = i * 1024
                for h in range(2):
                    nc.tensor.matmul(
                        ps[:, co + h * 512:co + (h + 1) * 512],
                        KTt[p][h * 64:(h + 1) * 64, kt * 128:(kt + 1) * 128],
                        QTt[p][h * 64:(h + 1) * 64, qb:qb + QC])
            eo = eop.tile([128, w], F16, tag=("eoA" if w == 2048 else "eoB"),
                          name="eo")
            nc.scalar.activation(eo[:], ps[:, 0:w], AF.Exp, scale=0.125)
            if pend is not None:
                emit_pv(p, caug, *pend)
            pend = (eo, kts)
            if si in hooks:
                for fn in hooks[si]:
                    fn()
        emit_pv(p, caug, *pend)
        # normalize: rows 64..127 of caug hold the softmax denominator
        ct = ctp.tile([128, QC], F16, tag="ct", name="ct")
        for h in range(2):
            rt = rp.tile([64, QC], F32, tag="rt", name="rt")
            nc.vector.tensor_copy(rt[:], caug[h][64:128, :])
            rt2 = rp.tile([64, QC], F32, tag="rt2", name="rt2")
            nc.vector.reciprocal_approx_fast(rt2[:], rt[:])
            nc.vector.tensor_mul(ct[h * 64:(h + 1) * 64, :], caug[h][0:64, :], rt2[:])
        return ct

    def outproj_piece(c, cts, qt):
        po = psp.tile([128, 1024], F32, tag="Bp", name=f"po{c}{qt}")
        for p in range(DT):
            for dmc in range(2):
                nc.tensor.matmul(po[:, dmc * 512:(dmc + 1) * 512],
                                 cts[p][:, qt * 128:(qt + 1) * 128],
                                 wo_sb[p][:, dmc * 512:(dmc + 1) * 512],
                                 start=(p == 0), stop=(p == DT - 1))
        ost = ostp.tile([128, 1024], F16, tag="ost", name="ost")
        nc.vector.tensor_copy(ost[:], po[:])
        (nc.sync if qt % 2 == 0 else nc.scalar).dma_start(
            t["rs_in"][c][qt * 128:(qt + 1) * 128, :], ost[:])

    def fire_rs(c):
        nc.gpsimd.collective_compute(
            "ReduceScatter", ALU.add, replica_groups=GROUPS,
            ins=[t["rs_in"][c].ap().opt()], outs=[t["rs_out"][c].ap().opt()])

    cts_prev = None
    c_prev = None
    for c in range(NCH):
        cts_cur = {}
        for p in range(DT):
            hooks = []
            if p == 0 and cts_prev is not None:
                cp, cc = c_prev, dict(cts_prev)
                hooks = [(1, [lambda cp=cp, cc=cc: outproj_piece(cp, cc, 0)]),
                         (3, [lambda cp=cp, cc=cc: outproj_piece(cp, cc, 1)]),
                         (5, [lambda cp=cp, cc=cc: outproj_piece(cp, cc, 2)]),
                         (7, [lambda cp=cp, cc=cc: outproj_piece(cp, cc, 3)]),
                         (9, [lambda cp=cp: fire_rs(cp)])]
            cts_cur[p] = sweep(c, p, hooks)
        cts_prev, c_prev = cts_cur, c

    # ---- tail: last chunk out-proj, RS, all LayerNorms ----
    for qt in range(4):
        outproj_piece(c_prev, cts_prev, qt)
    fire_rs(c_prev)
    for c in range(NCH):
        _layernorm_c(nc, t, c, lnp, stp, resid_sb, eps_t)


def _build():
    nc = bacc.Bacc("TRN2", target_bir_lowering=False, debug=False,
                   num_devices=N_CORES)
    t = {}
    for name in ("xqT", "xkT", "xvT"):
        t[name] = nc.dram_tensor(name, [DM, S], F16, kind="ExternalInput").ap()
    for name in ("wq", "wk", "wv"):
        t[name] = nc.dram_tensor(name, [DM, 256], F16, kind="ExternalInput").ap()
    t["wo"] = nc.dram_tensor("wo", [256, DM], F16, kind="ExternalInput").ap()
    t["resid"] = nc.dram_tensor("resid", [NCH, 128, DM], F32, kind="ExternalInput").ap()
    t["out"] = nc.dram_tensor("out", [NCH, 128, DM], F32, kind="ExternalOutput").ap()
    t["rs_in"] = [nc.dram_tensor(f"rs_in{c}", [512, DM], F16) for c in range(NCH)]
    t["rs_out"] = [nc.dram_tensor(f"rs_out{c}", [128, DM], F16) for c in range(NCH)]

    with tile.TileContext(nc) as tc:
        with ExitStack() as ctx:
            _emit(nc, tc, ctx, t)
    nc.compile()
    return nc


def kernel(input_Q, input_K, input_V, W_Q, W_K, W_V, W_O):
    global _NC
    if _NC is None:
        _NC = _build()
    nc = _NC

    input_Q = np.asarray(input_Q, dtype=np.float32)
    input_K = np.asarray(input_K, dtype=np.float32)
    input_V = np.asarray(input_V, dtype=np.float32)
    W_Q = np.asarray(W_Q, dtype=np.float32)
    W_K = np.asarray(W_K, dtype=np.float32)
    W_V = np.asarray(W_V, dtype=np.float32)
    W_O = np.asarray(W_O, dtype=np.float32)

    xT = {}
    for nm, x in (("q", input_Q), ("k", input_K), ("v", input_V)):
        for b in range(B):
            xT[nm, b] = np.ascontiguousarray(x[b].T).astype(np.float16)
    in_maps = []
    for core in range(N_CORES):
        b, j = core // 4, core % 4
        resid = np.empty((NCH, 128, DM), dtype=np.float32)
        for c in range(NCH):
            r0 = c * 512 + j * 128
            resid[c] = input_Q[b, r0:r0 + 128, :]
        in_maps.append({
            "xqT": xT["q", b], "xkT": xT["k", b], "xvT": xT["v", b],
            "wq": np.ascontiguousarray(W_Q[:, 256 * j:256 * j + 256]).astype(np.float16),
            "wk": np.ascontiguousarray(W_K[:, 256 * j:256 * j + 256]).astype(np.float16),
            "wv": np.ascontiguousarray(W_V[:, 256 * j:256 * j + 256]).astype(np.float16),
            "wo": np.ascontiguousarray(W_O[256 * j:256 * j + 256, :]).astype(np.float16),
            "resid": resid,
        })

    global _last_in_maps
    _last_in_maps = in_maps
    res = run_bass_kernel_spmd(nc, in_maps, core_ids=list(range(N_CORES)))

    out = np.empty((B, S, DM), dtype=np.float32)
    for core in range(N_CORES):
        b, j = core // 4, core % 4
        o = res.results[core]["out"]
        for c in range(NCH):
            r0 = c * 512 + j * 128
            out[b, r0:r0 + 128, :] = o[c]
    return out


# revision 11
# speedup vs baseline: 1.1544x; 1.1544x over previous
"""Trainium2 Bass kernel for nn_MultiHeadAttention_54211077210696.

8-core SPMD sharding: batch (2-way) x heads (4-way).
Core c (b = c//4, j = c%4) computes heads 4j..4j+3 of batch b.

v2 design:
  - Q^T/K^T projections in head-pair layout [128, S] (head h of pair at
    partitions h*64..h*64+64) -- no zero padding.
  - Scores via 64-contraction matmuls, row-tiled: the two heads of a pair
    run CONCURRENTLY on the PE array (tile_position (0,0)/(64,0) derived
    from base partitions), halving score time.
  - exp on ScalarE in wide instructions: PSUM slot A [128,2048] holds 2
    kt-tiles' scores (2 heads x 512q each), slot B [128,1024] holds 1.
    Pattern A,B,A,B,... decouples scores(kt+1) from exp(kt) (no WAR stall)
    while 2/3 of exp columns go through 2048-wide activations.
  - Softmax denominator from an all-ones block appended to V (rows 64..127
    of the PV output are the denominator).
  - q processed in 4 chunks of 512; out-projection + fp16 ReduceScatter
    fired per chunk, interleaved into the next chunk's score sweep so the
    collective tail is a single 1MB RS + one LayerNorm.
  - All LayerNorms deferred past the last exp (single ACT table switch).
  - Inputs prefetched on 5 DMA queues (sync/scalar/vector/tensor/gpsimd).
Matmuls fp16 with fp32 PSUM accumulation.
"""

import numpy as np
from contextlib import ExitStack

import concourse.bass as bass  # noqa: F401  (registers bass types)
import concourse.tile as tile
from concourse import bacc, mybir
from concourse.bass_utils import run_bass_kernel_spmd

F32 = mybir.dt.float32
F16 = mybir.dt.float16
AF = mybir.ActivationFunctionType
ALU = mybir.AluOpType

_NC = None

N_CORES = 8
GROUPS = [[0, 1, 2, 3], [4, 5, 6, 7]]
B, S, DM = 2, 2048, 1024
HL = 4          # heads per core
DT = 2          # head pairs per core
KT16 = 16       # 128-row k tiles
NCH = 4         # q chunks
QC = 512        # q chunk width
EPS = 1e-5


def _slot_plan():
    """Slot pattern for one (chunk, pair) sweep: [('A',[0,1]),('B',[2]),...]"""
    plan, k = [], 0
    while k < KT16:
        if len(plan) % 2 == 0:
            n = min(2, KT16 - k)
        else:
            n = 1
        plan.append(("AB"[len(plan) % 2], list(range(k, k + n))))
        k += n
    return plan


def _layernorm_c(nc, t, c, lnp, stp, resid_sb, eps_t):
    ro16 = lnp.tile([128, DM], F16, tag="ro16", name="ro16")
    nc.gpsimd.dma_start(ro16[:], t["rs_out"][c].ap())
    of = lnp.tile([128, DM], F32, tag="of", name="of")
    nc.vector.tensor_copy(of[:], ro16[:])
    orow = lnp.tile([128, DM], F32, tag="orow", name="orow")
    nc.vector.tensor_add(orow[:], of[:], resid_sb[c][:])
    stats = stp.tile([128, 2, 6], F32, tag="st", name="st")
    for i in range(2):
        nc.vector.bn_stats(stats[:, i, :], orow[:, i * 512:(i + 1) * 512])
    mv = stp.tile([128, 2], F32, tag="mv", name="mv")
    nc.vector.bn_aggr(mv[:], stats[:])
    rstd = stp.tile([128, 1], F32, tag="rstd", name="rstd")
    nc.scalar.activation(rstd[:], mv[:, 1:2], AF.Sqrt, bias=eps_t[:], scale=1.0)
    nc.vector.reciprocal(rstd[:], rstd[:])
    normed = lnp.tile([128, DM], F32, tag="norm", name="norm")
    nc.vector.tensor_scalar(normed[:], orow[:], mv[:, 0:1], rstd[:],
                            ALU.subtract, ALU.mult)
    nc.sync.dma_start(t["out"][c][:, 0:512], normed[:, 0:512])
    nc.scalar.dma_start(t["out"][c][:, 512:1024], normed[:, 512:1024])


def _emit(nc, tc, ctx, t):
    sing = ctx.enter_context(tc.tile_pool(name="sing", bufs=1))
    xp = ctx.enter_context(tc.tile_pool(name="xp", bufs=4))
    eop = ctx.enter_context(tc.tile_pool(name="eop", bufs=3))
    ctp = ctx.enter_context(tc.tile_pool(name="ctp", bufs=4))
    rp = ctx.enter_context(tc.tile_pool(name="rp", bufs=2))
    ostp = ctx.enter_context(tc.tile_pool(name="ostp", bufs=2))
    lnp = ctx.enter_context(tc.tile_pool(name="lnp", bufs=2))
    stp = ctx.enter_context(tc.tile_pool(name="stp", bufs=4))
    psp = ctx.enter_context(tc.tile_pool(name="psp", bufs=1, space="PSUM"))

    # persistent SBUF tiles
    QTt = [sing.tile([128, S], F16, tag=f"qt{d}", name=f"qt{d}") for d in range(DT)]
    KTt = [sing.tile([128, S], F16, tag=f"kt{d}", name=f"kt{d}") for d in range(DT)]
    vaug = sing.tile([128, HL * KT16 * 128], F16, tag="vaug")
    wo_sb = [sing.tile([128, DM], F16, tag=f"wo{p}", name=f"wo{p}") for p in range(DT)]
    resid_sb = [sing.tile([128, DM], F32, tag=f"res{c}", name=f"res{c}")
                for c in range(NCH)]
    eps_t = sing.tile([128, 1], F32, tag="eps")
    wkt = [sing.tile([128, 256], F16, tag=f"wk{i}", name=f"wk{i}") for i in range(8)]
    wqt = [sing.tile([128, 256], F16, tag=f"wq{i}", name=f"wq{i}") for i in range(8)]
    wvt = [sing.tile([128, 256], F16, tag=f"wv{i}", name=f"wv{i}") for i in range(8)]
    xv_res = [sing.tile([128, S], F16, tag=f"xv{i}", name=f"xv{i}") for i in range(8)]

    nc.vector.memset(eps_t[:], EPS)
    warm = sing.tile([128, 1], F32, tag="warm")
    nc.scalar.activation(warm[:], eps_t[:], AF.Exp, scale=1.0)
    nc.gpsimd.memset(vaug[:], 1.0)

    # ---- DMA prefetch on the 3 HWDGE queues (sync/scalar/gpsimd) ----
    # sync: wk, xk-lo, xq-lo (in qk_sweep), then rs_in/out DMAs
    # scalar: xk-hi, xq-hi, xv-hi
    # gpsimd: wq, wv, wo, xv-lo, resid
    for i in range(8):
        nc.sync.dma_start(wkt[i][:], t["wk"][i * 128:(i + 1) * 128, :])
    for i in range(8):
        nc.gpsimd.dma_start(wqt[i][:], t["wq"][i * 128:(i + 1) * 128, :])
    for i in range(8):
        nc.gpsimd.dma_start(wvt[i][:], t["wv"][i * 128:(i + 1) * 128, :])
    for p in range(DT):
        nc.gpsimd.dma_start(wo_sb[p][:], t["wo"][p * 128:(p + 1) * 128, :])
    for i in range(8):
        nc.gpsimd.dma_start(xv_res[i][:, 0:1024],
                            t["xvT"][i * 128:(i + 1) * 128, 0:1024])
    for c in range(NCH):
        nc.gpsimd.dma_start(resid_sb[c][:], t["resid"][c])

    # ---- K / Q projections: both d-tiles accumulate across all 8 banks ----
    # d0 -> A (4 chains), d1 -> B (2 chains) + 2 cg chains; all bank-aligned.
    def qk_sweep(x_dram, wt, dst, xtag, q1, q2):
        psA = psp.tile([128, 2048], F32, tag="A", name="psA")
        psB = psp.tile([128, 1024], F32, tag="Bp", name="psB")
        psc = [psp.tile([128, 512], F32, tag="cg", name="psc", bufs=2) for _ in range(2)]
        for dmc in range(8):
            xc = xp.tile([128, S], F16, tag=xtag, name="xc")
            q1.dma_start(xc[:, 0:1024], x_dram[dmc * 128:(dmc + 1) * 128, 0:1024])
            q2.dma_start(xc[:, 1024:2048], x_dram[dmc * 128:(dmc + 1) * 128, 1024:2048])
            st = dict(start=(dmc == 0), stop=(dmc == 7))
            for sc in range(4):
                nc.tensor.matmul(psA[:, sc * 512:(sc + 1) * 512],
                                 wt[dmc][:, 0:128], xc[:, sc * 512:(sc + 1) * 512], **st)
            for sc in range(4):
                out = (psB[:, sc * 512:(sc + 1) * 512] if sc < 2
                       else psc[sc - 2][:, :])
                nc.tensor.matmul(out, wt[dmc][:, 128:256],
                                 xc[:, sc * 512:(sc + 1) * 512], **st)
        for sc in range(4):
            nc.vector.tensor_copy(dst[0][:, sc * 512:(sc + 1) * 512],
                                  psA[:, sc * 512:(sc + 1) * 512])
        for sc in range(2):
            nc.vector.tensor_copy(dst[1][:, sc * 512:(sc + 1) * 512],
                                  psB[:, sc * 512:(sc + 1) * 512])
        for i in range(2):
            nc.vector.tensor_copy(dst[1][:, (2 + i) * 512:(3 + i) * 512], psc[i][:])

    qk_sweep(t["xkT"], wkt, KTt, "xk", nc.sync, nc.scalar)
    qk_sweep(t["xqT"], wqt, QTt, "xq", nc.sync, nc.scalar)

    # xv high halves ride the scalar queue after the K/Q chunks
    for i in range(8):
        nc.scalar.dma_start(xv_res[i][:, 1024:2048],
                            t["xvT"][i * 128:(i + 1) * 128, 1024:2048])

    # ---- V projection: 2 waves x 8 s-tile chains across all 8 banks ----
    for wave in range(2):
        psA = psp.tile([128, 2048], F32, tag="A", name="psAv")
        psB = psp.tile([128, 1024], F32, tag="Bp", name="psBv")
        psc = [psp.tile([128, 512], F32, tag="cg", name="pscv", bufs=2) for _ in range(2)]

        def vchain(i):
            if i < 4:
                return psA[:, i * 512:i * 512 + 256]
            if i < 6:
                return psB[:, (i - 4) * 512:(i - 4) * 512 + 256]
            return psc[i - 6][:, 0:256]

        for dmc in range(8):
            for i in range(8):
                st = wave * 8 + i
                nc.tensor.matmul(vchain(i), xv_res[dmc][:, st * 128:(st + 1) * 128],
                                 wvt[dmc][:], start=(dmc == 0), stop=(dmc == 7))
        for i in range(8):
            st = wave * 8 + i
            src = vchain(i)
            for h in range(HL):
                nc.vector.tensor_copy(
                    vaug[:, (h * KT16 + st) * 128:(h * KT16 + st) * 128 + 64],
                    src[:, h * 64:h * 64 + 64])

    # ---- attention: 4 chunks x 2 pairs, slot pattern A(2kt) B(1kt) ----
    plan = _slot_plan()

    def emit_pv(p, caug, eo, kts):
        for i, kt in enumerate(kts):
            for h in range(2):
                blk = ((p * 2 + h) * KT16 + kt) * 128
                nc.tensor.matmul(caug[h][:, :], vaug[:, blk:blk + 128],
                                 eo[:, i * 1024 + h * 512:i * 1024 + (h + 1) * 512],
                                 start=(kt == 0), stop=(kt == KT16 - 1))

    def sweep(c, p, hooks):
        qb = c * QC
        caug = [psp.tile([128, QC], F32, tag="cg", name=f"caug{h}", bufs=2) for h in range(2)]
        pend = []          # PV deferred by 2 slots so scores stay ahead on PE
        hooks = dict(hooks)
        for si, (kind, kts) in enumerate(plan):
            w = 1024 * len(kts)
            ps = psp.tile([128, 2048 if kind == "A" else 1024], F32,
                          tag=("A" if kind == "A" else "Bp"), name="pss")
            for i, kt in enumerate(kts):
                co = i * 1024
                for h in range(2):
                    nc.tensor.matmul(
                        ps[:, co + h * 512:co + (h + 1) * 512],
                        KTt[p][h * 64:(h + 1) * 64, kt * 128:(kt + 1) * 128],
                        QTt[p][h * 64:(h + 1) * 64, qb:qb + QC])
            eo = eop.tile([128, w], F16, tag=("eoA" if w == 2048 else "eoB"),
                          name="eo")
            nc.scalar.activation(eo[:], ps[:, 0:w], AF.Exp, scale=0.125)
            pend.append((eo, kts))
            if len(pend) > 2:
                emit_pv(p, caug, *pend.pop(0))
            if si in hooks:
                for fn in hooks[si]:
                    fn()
        for pe in pend:
            emit_pv(p, caug, *pe)
        # normalize: rows 64..127 of caug hold the softmax denominator
        ct = ctp.tile([128, QC], F16, tag="ct", name="ct")
        for h in range(2):
            rt = rp.tile([64, QC], F32, tag="rt", name="rt")
            nc.vector.tensor_copy(rt[:], caug[h][64:128, :])
            rt2 = rp.tile([64, QC], F32, tag="rt2", name="rt2")
            nc.vector.reciprocal_approx_fast(rt2[:], rt[:])
            nc.vector.tensor_mul(ct[h * 64:(h + 1) * 64, :], caug[h][0:64, :], rt2[:])
        return ct

    def outproj_piece(c, cts, qt):
        po = psp.tile([128, 1024], F32, tag="Bp", name=f"po{c}{qt}")
        for p in range(DT):
            for dmc in range(2):
                nc.tensor.matmul(po[:, dmc * 512:(dmc + 1) * 512],
                                 cts[p][:, qt * 128:(qt + 1) * 128],
                                 wo_sb[p][:, dmc * 512:(dmc + 1) * 512],
                                 start=(p == 0), stop=(p == DT - 1))
        ost = ostp.tile([128, 1024], F16, tag="ost", name="ost")
        nc.vector.tensor_copy(ost[:], po[:])
        (nc.sync if qt % 2 == 0 else nc.scalar).dma_start(
            t["rs_in"][c][qt * 128:(qt + 1) * 128, :], ost[:])

    def fire_rs(c):
        nc.gpsimd.collective_compute(
            "ReduceScatter", ALU.add, replica_groups=GROUPS,
            ins=[t["rs_in"][c].ap().opt()], outs=[t["rs_out"][c].ap().opt()])

    cts_prev = None
    c_prev = None
    for c in range(NCH):
        cts_cur = {}
        for p in range(DT):
            hooks = []
            if p == 0 and cts_prev is not None:
                cp, cc = c_prev, dict(cts_prev)
                hooks = [(1, [lambda cp=cp, cc=cc: outproj_piece(cp, cc, 0)]),
                         (3, [lambda cp=cp, cc=cc: outproj_piece(cp, cc, 1)]),
                         (5, [lambda cp=cp, cc=cc: outproj_piece(cp, cc, 2)]),
                         (7, [lambda cp=cp, cc=cc: outproj_piece(cp, cc, 3)]),
                         (9, [lambda cp=cp: fire_rs(cp)])]
            cts_cur[p] = sweep(c, p, hooks)
        cts_prev, c_prev = cts_cur, c

    # ---- tail: last chunk out-proj, RS, all LayerNorms ----
    for qt in range(4):
        outproj_piece(c_prev, cts_prev, qt)
    fire_rs(c_prev)
    # tile_wait_until pins the LNs to the end of every engine queue: the
    # scheduler must not hoist the Sqrt (ACT table switch) or the rs_out
    # reads into the loop, where they would stall behind the collectives.
    for c in range(NCH):
        with tc.tile_wait_until(2.0 + 0.01 * c):
            _layernorm_c(nc, t, c, lnp, stp, resid_sb, eps_t)


def _build():
    nc = bacc.Bacc("TRN2", target_bir_lowering=False, debug=False,
                   num_devices=N_CORES)
    t = {}
    for name in ("xqT", "xkT", "xvT"):
        t[name] = nc.dram_tensor(name, [DM, S], F16, kind="ExternalInput").ap()
    for name in ("wq", "wk", "wv"):
        t[name] = nc.dram_tensor(name, [DM, 256], F16, kind="ExternalInput").ap()
    t["wo"] = nc.dram_tensor("wo", [256, DM], F16, kind="ExternalInput").ap()
    t["resid"] = nc.dram_tensor("resid", [NCH, 128, DM], F32, kind="ExternalInput").ap()
    t["out"] = nc.dram_tensor("out", [NCH, 128, DM], F32, kind="ExternalOutput").ap()
    t["rs_in"] = [nc.dram_tensor(f"rs_in{c}", [512, DM], F16) for c in range(NCH)]
    t["rs_out"] = [nc.dram_tensor(f"rs_out{c}", [128, DM], F16) for c in range(NCH)]

    with tile.TileContext(nc) as tc:
        with ExitStack() as ctx:
            _emit(nc, tc, ctx, t)
    nc.compile()
    return nc


def kernel(input_Q, input_K, input_V, W_Q, W_K, W_V, W_O):
    global _NC
    if _NC is None:
        _NC = _build()
    nc = _NC

    input_Q = np.asarray(input_Q, dtype=np.float32)
    input_K = np.asarray(input_K, dtype=np.float32)
    input_V = np.asarray(input_V, dtype=np.float32)
    W_Q = np.asarray(W_Q, dtype=np.float32)
    W_K = np.asarray(W_K, dtype=np.float32)
    W_V = np.asarray(W_V, dtype=np.float32)
    W_O = np.asarray(W_O, dtype=np.float32)

    xT = {}
    for nm, x in (("q", input_Q), ("k", input_K), ("v", input_V)):
        for b in range(B):
            xT[nm, b] = np.ascontiguousarray(x[b].T).astype(np.float16)
    in_maps = []
    for core in range(N_CORES):
        b, j = core // 4, core % 4
        resid = np.empty((NCH, 128, DM), dtype=np.float32)
        for c in range(NCH):
            r0 = c * 512 + j * 128
            resid[c] = input_Q[b, r0:r0 + 128, :]
        in_maps.append({
            "xqT": xT["q", b], "xkT": xT["k", b], "xvT": xT["v", b],
            "wq": np.ascontiguousarray(W_Q[:, 256 * j:256 * j + 256]).astype(np.float16),
            "wk": np.ascontiguousarray(W_K[:, 256 * j:256 * j + 256]).astype(np.float16),
            "wv": np.ascontiguousarray(W_V[:, 256 * j:256 * j + 256]).astype(np.float16),
            "wo": np.ascontiguousarray(W_O[256 * j:256 * j + 256, :]).astype(np.float16),
            "resid": resid,
        })

    global _last_in_maps
    _last_in_maps = in_maps
    res = run_bass_kernel_spmd(nc, in_maps, core_ids=list(range(N_CORES)))

    out = np.empty((B, S, DM), dtype=np.float32)
    for core in range(N_CORES):
        b, j = core // 4, core % 4
        o = res.results[core]["out"]
        for c in range(NCH):
            r0 = c * 512 + j * 128
            out[b, r0:r0 + 128, :] = o[c]
    return out
